# revision 26
# baseline (speedup 1.0000x reference)
"""Trainium2 Bass kernel for nn_BinaryLutLayer (embedding_lookup).

Per output row n (of 16384): addr = sum_b x[n,b] * 2^b (14 bits), then
y[n] = float32(luts_int[n, addr]).

Sharding: rows split across 8 cores (2048 rows each), no communication.

Per-core pipeline (raw Bass, hand-scheduled across 5 engines):
  DVE:    hi = addr>>8 as a direct bit-weighted reduce of x,
          k16 = (addr>>1)&127 and parity the same way; is_equal mask
          select of the gathered halfword; exact bitwise byte extract.
  PE:     one transpose + one tiled-identity matmul land the int16
          block indices in the wrapped layout the gather firmware
          expects (partition q%16, col q//16, replicated to all 8
          gpsimd cores).
  Pool:   4x dma_gather - 512 rows' 256B LUT blocks each, 2048
          descriptors total instead of reading the 32 MB LUT shard.
  SP/ACT: input/output DMAs on parallel HWDGE queues.

The host does layout-only work: one combined x tensor (bits 0..7 in
select-slot layout, bits 8..13 in transpose-friendly layout), LUT chunk
slicing, and un-permuting y. All integer arithmetic on device is exact:
fp32 adds below 2^24 and bitwise ALU ops only.
"""

import numpy as np

NUM_BITS = 14
NUM_OUT = 16384
LUT_SIZE = 2**NUM_BITS
CORES = 8
NS = NUM_OUT // CORES  # rows per core = 2048
P = 128  # SBUF partitions
T = NS // P  # row-slots per partition = 16
NCHUNK = 4
CHUNK = NS // NCHUNK  # rows per dma_gather = 512
BLK = 256  # gather element size (bytes)
NBLK = CHUNK * (LUT_SIZE // BLK)  # blocks per LUT chunk = 32768
NCOL = 525  # consts columns

_CACHE: dict = {}


def _build_nc():
    import concourse.bacc as bacc
    from concourse import bass, mybir

    f32, i32, i16, i8, u16, u32 = (
        mybir.dt.float32,
        mybir.dt.int32,
        mybir.dt.int16,
        mybir.dt.int8,
        mybir.dt.uint16,
        mybir.dt.uint32,
    )
    Alu = mybir.AluOpType
    X = mybir.AxisListType.X

    nc = bacc.Bacc(
        "TRN2",
        target_bir_lowering=False,
        debug=False,
        dynamic_dma_scratch_size=65536,
        num_swdge_queues=4,
    )

    x_t = nc.dram_tensor("x_shard", [NS, NUM_BITS], f32, kind="ExternalInput")
    lut_t = [
        nc.dram_tensor(f"lut{c}", [NBLK, BLK], i8, kind="ExternalInput")
        for c in range(NCHUNK)
    ]
    co_t = nc.dram_tensor("consts", [P, NCOL], f32, kind="ExternalInput")
    y_t = nc.dram_tensor("y_shard", [NS, 1], f32, kind="ExternalOutput")

    from contextlib import ExitStack

    with ExitStack() as ctx:
        dx = ctx.enter_context(nc.semaphore("dx"))
        dc = ctx.enter_context(nc.semaphore("dc"))
        vd = ctx.enter_context(nc.semaphore("vd"))
        ps = ctx.enter_context(nc.semaphore("psem"))
        gsl = [ctx.enter_context(nc.semaphore(f"gs{i}")) for i in range(NCHUNK)]
        gw = ctx.enter_context(nc.semaphore("gw"))
        dy = ctx.enter_context(nc.semaphore("dy"))
        widx = ctx.enter_context(nc.sbuf_tensor("widx", [P, 1], i16))
        wout = ctx.enter_context(nc.sbuf_tensor("wout", [P, BLK], i8))
        x_sb = ctx.enter_context(nc.sbuf_tensor("x_sb", [P, T * NUM_BITS], f32))
        co_sb = ctx.enter_context(nc.sbuf_tensor("co_sb", [P, NCOL], f32))
        prodh = ctx.enter_context(nc.sbuf_tensor("prodh", [P, T * 6], f32))
        prodk = ctx.enter_context(nc.sbuf_tensor("prodk", [P, T * 6], f32))
        hi2_f = ctx.enter_context(nc.sbuf_tensor("hi2_f", [P, T], f32))
        hiT_ps = ctx.enter_context(nc.psum_tensor("hiT_ps", [16, P], f32))
        hiT_sb = ctx.enter_context(nc.sbuf_tensor("hiT_sb", [16, P], f32))
        rep_ps = ctx.enter_context(nc.psum_tensor("rep_ps", [P, P], f32))
        idxw = ctx.enter_context(nc.sbuf_tensor("idxw", [P, P], i16))
        blocks = ctx.enter_context(nc.sbuf_tensor("blocks", [P, T * BLK], i8))
        k16_f = ctx.enter_context(nc.sbuf_tensor("k16_f", [P, T], f32))
        k32_u = ctx.enter_context(nc.sbuf_tensor("k32_u", [P, T], u32))
        iota32 = ctx.enter_context(nc.sbuf_tensor("iota32", [P, BLK // 4], u32))
        tmp8 = ctx.enter_context(nc.sbuf_tensor("tmp8", [P, T], i32))
        shmt = ctx.enter_context(nc.sbuf_tensor("shmt", [P, T], i32))
        mask = ctx.enter_context(nc.sbuf_tensor("mask", [P, T * (BLK // 4)], i32))
        msel = ctx.enter_context(nc.sbuf_tensor("msel", [P, T * (BLK // 4)], i32))
        y32u = ctx.enter_context(nc.sbuf_tensor("y32u", [P, T], i32))
        sh_i = ctx.enter_context(nc.sbuf_tensor("sh_i", [P, T], i32))
        u8_i = ctx.enter_context(nc.sbuf_tensor("u8_i", [P, T], i32))
        y_f = ctx.enter_context(nc.sbuf_tensor("y_f", [P, T], f32))
        w17 = co_sb[:, 0:7]  # 2^(b-1), b=1..7
        wh = co_sb[:, 7:13]  # 2^(b-8), b=8..13
        iota = co_sb[:, 13:141]  # value k, f32
        ident = co_sb[:, 141:269]
        qw16 = co_sb[0:16, 269:397]  # q*64 at its wrap position
        etile = co_sb[0:16, 397:525]  # E[k, m] = (m%16 == k)

        x3 = x_sb[:].rearrange("p (t b) -> p t b", b=NUM_BITS)
        ph3 = prodh[:].rearrange("p (t b) -> p t b", b=6)
        pk3 = prodk[:].rearrange("p (t b) -> p t b", b=6)
        wh3 = wh.rearrange("p b -> p () b").to_broadcast([P, T, 6])
        w173 = w17[:, 0:6].rearrange("p b -> p () b").to_broadcast([P, T, 6])
        blocks4 = blocks[:].rearrange("p (c j k) -> p c j k", c=NCHUNK, k=BLK)
        blocks_i32 = blocks[:].bitcast(i32).rearrange(
            "p (c j k) -> p c j k", c=NCHUNK, k=BLK // 4
        )
        mask4 = mask[:].rearrange("p (c j k) -> p c j k", c=NCHUNK, k=BLK // 4)
        msel4 = msel[:].rearrange("p (c j k) -> p c j k", c=NCHUNK, k=BLK // 4)
        iota32_b = iota32[:].rearrange("p k -> p () k").to_broadcast(
            [P, NCHUNK, BLK // 4]
        )
        k32_4 = k32_u[:].rearrange("p (c j) -> p c j", c=NCHUNK)
        # dispatch order: a gather whose Q7 core pair differs from the
        # previous gather's pair dispatches ~instantly, so queue 0 (the
        # warmup's pair) goes last and all four generations overlap.
        # (queue, chunk, j0, j1) in dispatch order. Queue q runs on Q7 pair
        # (2q, 2q+1); pair 0 is ~2.2x slower AND blocks the next dispatch
        # until it finishes, so queue 0 gets the least work and goes last.
        PIECES = [
            (1, 0, 0, 2), (2, 1, 0, 2), (3, 2, 0, 2),
            (1, 3, 0, 3), (2, 1, 2, 4), (3, 2, 2, 4),
            (0, 3, 3, 4), (0, 0, 2, 4),
        ]
        WAITS = []
        _cnt = {0: 0, 1: 0, 2: 0, 3: 0}
        for _q, _c, _j0, _j1 in PIECES:
            _cnt[_q] += 16
            WAITS.append(_cnt[_q])

        with nc.Block(no_gpsimd_drain=False) as block:

            @block.sync
            def _(s):
                s.dma_start(
                    x_sb[:], x_t[:].rearrange("(p t) b -> p (t b)", p=P)
                ).then_inc(dx, 16)
                s.wait_ge(vd, 37)  # y_f ready
                s.dma_start(
                    y_t[:].rearrange("(p t) one -> p (t one)", p=P), y_f[:]
                ).then_inc(dy, 16)
                s.wait_ge(dy, 16)

            @block.scalar
            def _(s):
                s.dma_start(co_sb[:], co_t[:]).then_inc(dc, 16)

            @block.tensor
            def _(t):
                t.wait_ge(vd, 2)  # hi2_f ready (implies consts loaded)
                t.transpose(out=hiT_ps[:], in_=hi2_f[:], identity=ident).then_inc(
                    ps, 1
                )
                t.wait_ge(vd, 6)  # hiT_sb (= hiT + q*64) ready
                t.matmul(rep_ps[:], lhsT=etile, rhs=hiT_sb[:]).then_inc(ps, 1)

            @block.gpsimd
            def _(g):
                # warm-up: the first dma_gather on a fresh NEFF stalls ~10us
                # (custom-op ucode IRAM load on all 8 Q7 cores). num_idxs=0
                # (all-negative idx) hangs the DMA, so keep 16 real indices.
                g.memset(widx[:], 0).then_inc(gw, 1)
                g.wait_ge(gw, 1)
                g.dma_gather(
                    out_ap=wout[:].rearrange("p (j k) -> p j k", k=BLK),
                    in_ap=lut_t[0][:],
                    idxs_ap=widx[:],
                    num_idxs=16,
                    num_idxs_reg=16,
                    elem_size=BLK,
                ).then_inc(gw, 16)
                g.wait_ge(vd, 7)  # idxw ready
                # queue c -> Q7 core pair (2c, 2c+1): descriptor generation
                # runs on four disjoint core pairs; each chunk is split in
                # two so the first half's data lands (and DVE select starts)
                # while the second half still generates.
                for q, c, j0, j1 in PIECES:
                    nidx = (j1 - j0) * 128
                    g.dma_gather(
                        out_ap=blocks4[:, c, j0:j1],
                        in_ap=lut_t[c][:],
                        idxs_ap=idxw[:, c * 32 + 8 * j0 : c * 32 + 8 * j1],
                        num_idxs=nidx,
                        num_idxs_reg=nidx,
                        elem_size=BLK,
                        queue_num=q,
                    ).then_inc(gsl[q], 16)

            @block.vector
            def _(v):
                # the DVE pipeline is not hazard-safe for back-to-back
                # dependent ops: chain every op through sem `vd`
                n = [0]

                def step(inst):
                    inst.then_inc(vd, 1)
                    n[0] += 1

                def w():
                    if n[0]:
                        v.wait_ge(vd, n[0])

                v.wait_ge(dx, 16)
                v.wait_ge(dc, 16)
                # hi = addr>>8 directly from the high bits of x
                step(v.tensor_tensor(out=ph3, in0=x3[:, :, 8:14], in1=wh3, op=Alu.mult))
                w()
                step(v.tensor_reduce(out=hi2_f[:], in_=ph3, axis=X, op=Alu.add))
                # select-path arithmetic fills the PE-transpose latency
                step(v.tensor_tensor(out=pk3, in0=x3[:, :, 2:8], in1=w173, op=Alu.mult))
                w()
                step(v.tensor_reduce(out=k16_f[:], in_=pk3, axis=X, op=Alu.add))
                step(v.tensor_scalar(
                    out=tmp8[:],
                    in0=x3[:, :, 0:1].rearrange("p t one -> p (t one)"),
                    scalar1=8.0, scalar2=None, op0=Alu.mult,
                ))
                v.wait_ge(ps, 1)
                step(v.tensor_tensor(
                    out=hiT_sb[:], in0=hiT_ps[:], in1=qw16, op=Alu.add
                ))
                v.wait_ge(ps, 2)
                step(v.tensor_copy(out=idxw[:], in_=rep_ps[:]))
                # u32 select state: k32 = (addr>>2)&63, iota 0..63,
                # shmt = 8*(addr&3) for the final byte shift
                w()
                step(v.tensor_copy(out=k32_u[:], in_=k16_f[:]))
                step(v.tensor_copy(out=iota32[:], in_=iota[:, 0 : BLK // 4]))
                step(v.scalar_tensor_tensor(
                    out=shmt[:],
                    in0=x3[:, :, 1:2].rearrange("p t one -> p (t one)"),
                    scalar=16.0, in1=tmp8[:], op0=Alu.mult, op1=Alu.add,
                ))
                # masks don't depend on the gathers
                w()
                for c in range(NCHUNK):
                    kb = (
                        k32_4[:, c]
                        .rearrange("p j -> p j ()")
                        .to_broadcast([P, NCHUNK, BLK // 4])
                    )
                    step(v.tensor_tensor(
                        out=mask4[:, c], in0=iota32_b, in1=kb, op=Alu.not_equal
                    ))
                for c in range(NCHUNK):
                    w()
                    step(v.tensor_scalar(
                        out=mask4[:, c], in0=mask4[:, c], scalar1=1,
                        scalar2=None, op0=Alu.subtract,
                    ))
                for (q, c, j0, j1), wcnt in zip(PIECES, WAITS):
                    v.wait_ge(gsl[q], wcnt)
                    w()
                    step(v.tensor_tensor(
                        out=msel4[:, c, j0:j1],
                        in0=mask4[:, c, j0:j1],
                        in1=blocks_i32[:, c, j0:j1],
                        op=Alu.bitwise_and,
                    ))
                    w()
                    # one -1 mask per row selects its u32; OR-reduce is
                    # bit-exact for any int8 LUT content
                    step(v.tensor_reduce(
                        out=y32u[:, c * NCHUNK + j0 : c * NCHUNK + j1],
                        in_=msel4[:, c, j0:j1],
                        axis=X, op=Alu.bitwise_or,
                    ))
                # byte extract + sign-extend, exact bitwise ops
                w()
                step(v.tensor_tensor(
                    out=sh_i[:], in0=y32u[:], in1=shmt[:],
                    op=Alu.logical_shift_right,
                ))
                w()
                step(v.tensor_scalar(
                    out=u8_i[:], in0=sh_i[:], scalar1=255, scalar2=128,
                    op0=Alu.bitwise_and, op1=Alu.bitwise_xor,
                ))
                w()
                step(v.tensor_scalar(
                    out=y_f[:], in0=u8_i[:], scalar1=128, scalar2=None,
                    op0=Alu.subtract,
                ))  # vd -> 37: y_f ready

    nc.compile()
    return nc


def _get_nc():
    if "nc" not in _CACHE:
        _CACHE["nc"] = _build_nc()
    return _CACHE["nc"]


def _consts() -> np.ndarray:
    co = np.zeros((P, NCOL), dtype=np.float32)
    co[:, 0:7] = 2.0 ** np.arange(0, 7, dtype=np.float32)  # 2^(b-1), b=1..7
    co[:, 7:13] = 2.0 ** np.arange(0, 6, dtype=np.float32)  # 2^(b-8), b=8..13
    co[:, 13:141] = np.arange(P, dtype=np.float32)[None, :]
    co[:, 141:269] = np.eye(P, dtype=np.float32)
    # qw16[qh, pi] = (j*128 + u*16 + qh) * 64 with pi = c*32 + j*8 + u
    pi = np.arange(P)
    j, u = (pi % 32) // 8, pi % 8
    co[:, 269:397] = (
        (j * P + u * 16)[None, :] + (np.arange(P) % 16)[:, None]
    ).astype(np.float32) * 64.0
    co[:16, 397:525] = (
        (np.arange(P)[None, :] % 16) == np.arange(16)[:, None]
    ).astype(np.float32)
    return co


def _make_in_maps(x, luts_int):
    co = _consts()
    x = np.asarray(x, dtype=np.float32).reshape(NUM_OUT, NUM_BITS)
    luts_int = np.asarray(luts_int, dtype=np.int8)
    in_maps = []
    for core in range(CORES):
        base = core * NS
        xl = x[base : base + NS]
        # combined layout [p, tau, b]: bits 0..7 from select-slot layout
        # (row tau*128+p), bits 8..13 from transpose layout (row p*16+tau)
        xs = np.empty((P, T, NUM_BITS), dtype=np.float32)
        xs[:, :, 0:8] = xl.reshape(T, P, NUM_BITS).transpose(1, 0, 2)[:, :, 0:8]
        xs[:, :, 8:14] = xl.reshape(P, T, NUM_BITS)[:, :, 8:14]
        m = {"x_shard": xs.reshape(NS, NUM_BITS), "consts": co}
        for c in range(NCHUNK):
            m[f"lut{c}"] = luts_int[
                base + c * CHUNK : base + (c + 1) * CHUNK
            ].reshape(NBLK, BLK)
        in_maps.append(m)
    return in_maps


def kernel(x, luts_float, luts_int, _run_kwargs=None):
    from concourse.bass_utils import run_bass_kernel_spmd

    nc = _get_nc()
    in_maps = _make_in_maps(x, luts_int)
    res = run_bass_kernel_spmd(nc, in_maps, list(range(CORES)), **(_run_kwargs or {}))
    _CACHE["last_result"] = res
    out = np.empty((NUM_OUT, 1), dtype=np.float32)
    for core in range(CORES):
        ys = res.results[core]["y_shard"].reshape(P, T)  # [p, t]
        out[core * NS : (core + 1) * NS, 0] = ys.T.reshape(NS)
    return out



# revision 27
# speedup vs baseline: 1.0153x; 1.0153x over previous
"""Trainium2 Bass kernel for nn_BinaryLutLayer (embedding_lookup).

Per output row n (of 16384): addr = sum_b x[n,b] * 2^b (14 bits), then
y[n] = float32(luts_int[n, addr]).

Sharding: rows split across 8 cores (2048 rows each), no communication.

Per-core pipeline (raw Bass, hand-scheduled across 5 engines):
  DVE:    hi = addr>>8 as a direct bit-weighted reduce of x,
          k16 = (addr>>1)&127 and parity the same way; is_equal mask
          select of the gathered halfword; exact bitwise byte extract.
  PE:     one transpose + one tiled-identity matmul land the int16
          block indices in the wrapped layout the gather firmware
          expects (partition q%16, col q//16, replicated to all 8
          gpsimd cores).
  Pool:   4x dma_gather - 512 rows' 256B LUT blocks each, 2048
          descriptors total instead of reading the 32 MB LUT shard.
  SP/ACT: input/output DMAs on parallel HWDGE queues.

The host does layout-only work: one combined x tensor (bits 0..7 in
select-slot layout, bits 8..13 in transpose-friendly layout), LUT chunk
slicing, and un-permuting y. All integer arithmetic on device is exact:
fp32 adds below 2^24 and bitwise ALU ops only.
"""

import numpy as np

NUM_BITS = 14
NUM_OUT = 16384
LUT_SIZE = 2**NUM_BITS
CORES = 8
NS = NUM_OUT // CORES  # rows per core = 2048
P = 128  # SBUF partitions
T = NS // P  # row-slots per partition = 16
NCHUNK = 4
CHUNK = NS // NCHUNK  # rows per dma_gather = 512
BLK = 256  # gather element size (bytes)
NBLK = CHUNK * (LUT_SIZE // BLK)  # blocks per LUT chunk = 32768
NCOL = 525  # consts columns

_CACHE: dict = {}


def _build_nc():
    import concourse.bacc as bacc
    from concourse import bass, mybir

    f32, i32, i16, i8, u16, u32 = (
        mybir.dt.float32,
        mybir.dt.int32,
        mybir.dt.int16,
        mybir.dt.int8,
        mybir.dt.uint16,
        mybir.dt.uint32,
    )
    Alu = mybir.AluOpType
    X = mybir.AxisListType.X

    nc = bacc.Bacc(
        "TRN2",
        target_bir_lowering=False,
        debug=False,
        dynamic_dma_scratch_size=65536,
        num_swdge_queues=4,
    )

    x_t = nc.dram_tensor("x_shard", [NS, NUM_BITS], f32, kind="ExternalInput")
    lut_t = [
        nc.dram_tensor(f"lut{c}", [NBLK, BLK], i8, kind="ExternalInput")
        for c in range(NCHUNK)
    ]
    co_t = nc.dram_tensor("consts", [P, NCOL], f32, kind="ExternalInput")
    y_t = nc.dram_tensor("y_shard", [NS, 1], f32, kind="ExternalOutput")

    from contextlib import ExitStack

    with ExitStack() as ctx:
        dx = ctx.enter_context(nc.semaphore("dx"))
        dc = ctx.enter_context(nc.semaphore("dc"))
        vd = ctx.enter_context(nc.semaphore("vd"))
        ps = ctx.enter_context(nc.semaphore("psem"))
        gsl = [ctx.enter_context(nc.semaphore(f"gs{i}")) for i in range(NCHUNK)]
        gw = ctx.enter_context(nc.semaphore("gw"))
        dy = ctx.enter_context(nc.semaphore("dy"))
        widx = ctx.enter_context(nc.sbuf_tensor("widx", [P, 1], i16))
        wout = ctx.enter_context(nc.sbuf_tensor("wout", [P, BLK], i8))
        x_sb = ctx.enter_context(nc.sbuf_tensor("x_sb", [P, T * NUM_BITS], f32))
        co_sb = ctx.enter_context(nc.sbuf_tensor("co_sb", [P, NCOL], f32))
        prodh = ctx.enter_context(nc.sbuf_tensor("prodh", [P, T * 6], f32))
        prodk = ctx.enter_context(nc.sbuf_tensor("prodk", [P, T * 6], f32))
        hi2_f = ctx.enter_context(nc.sbuf_tensor("hi2_f", [P, T], f32))
        hiT_ps = ctx.enter_context(nc.psum_tensor("hiT_ps", [16, P], f32))
        hiT_sb = ctx.enter_context(nc.sbuf_tensor("hiT_sb", [16, P], f32))
        rep_ps = ctx.enter_context(nc.psum_tensor("rep_ps", [P, P], f32))
        idxw = ctx.enter_context(nc.sbuf_tensor("idxw", [P, P], i16))
        blocks = ctx.enter_context(nc.sbuf_tensor("blocks", [P, T * BLK], i8))
        k16_f = ctx.enter_context(nc.sbuf_tensor("k16_f", [P, T], f32))
        k32_u = ctx.enter_context(nc.sbuf_tensor("k32_u", [P, T], u32))
        iota32 = ctx.enter_context(nc.sbuf_tensor("iota32", [P, BLK // 4], u32))
        tmp8 = ctx.enter_context(nc.sbuf_tensor("tmp8", [P, T], i32))
        shmt = ctx.enter_context(nc.sbuf_tensor("shmt", [P, T], i32))
        mask = ctx.enter_context(nc.sbuf_tensor("mask", [P, T * (BLK // 4)], i32))
        msel = ctx.enter_context(nc.sbuf_tensor("msel", [P, T * (BLK // 4)], i32))
        y32u = ctx.enter_context(nc.sbuf_tensor("y32u", [P, T], i32))
        sh_i = ctx.enter_context(nc.sbuf_tensor("sh_i", [P, T], i32))
        u8_i = ctx.enter_context(nc.sbuf_tensor("u8_i", [P, T], i32))
        y_f = ctx.enter_context(nc.sbuf_tensor("y_f", [P, T], f32))
        w17 = co_sb[:, 0:7]  # 2^(b-1), b=1..7
        wh = co_sb[:, 7:13]  # 2^(b-8), b=8..13
        iota = co_sb[:, 13:141]  # value k, f32
        ident = co_sb[:, 141:269]
        qw16 = co_sb[0:16, 269:397]  # q*64 at its wrap position
        etile = co_sb[0:16, 397:525]  # E[k, m] = (m%16 == k)

        x3 = x_sb[:].rearrange("p (t b) -> p t b", b=NUM_BITS)
        ph3 = prodh[:].rearrange("p (t b) -> p t b", b=6)
        pk3 = prodk[:].rearrange("p (t b) -> p t b", b=6)
        wh3 = wh.rearrange("p b -> p () b").to_broadcast([P, T, 6])
        w173 = w17[:, 0:6].rearrange("p b -> p () b").to_broadcast([P, T, 6])
        blocks4 = blocks[:].rearrange("p (c j k) -> p c j k", c=NCHUNK, k=BLK)
        blocks_i32 = blocks[:].bitcast(i32).rearrange(
            "p (c j k) -> p c j k", c=NCHUNK, k=BLK // 4
        )
        mask4 = mask[:].rearrange("p (c j k) -> p c j k", c=NCHUNK, k=BLK // 4)
        msel4 = msel[:].rearrange("p (c j k) -> p c j k", c=NCHUNK, k=BLK // 4)
        iota32_b = iota32[:].rearrange("p k -> p () k").to_broadcast(
            [P, NCHUNK, BLK // 4]
        )
        k32_4 = k32_u[:].rearrange("p (c j) -> p c j", c=NCHUNK)
        # dispatch order: a gather whose Q7 core pair differs from the
        # previous gather's pair dispatches ~instantly, so queue 0 (the
        # warmup's pair) goes last and all four generations overlap.
        CORDER = [1, 2, 3, 0]
        # (chunk/queue, half) dispatch order; queue 0 last
        DORDER = [(1, 0), (2, 0), (3, 0), (1, 1), (2, 1), (3, 1), (0, 0), (0, 1)]

        with nc.Block(no_gpsimd_drain=False) as block:

            @block.sync
            def _(s):
                s.dma_start(
                    x_sb[:], x_t[:].rearrange("(p t) b -> p (t b)", p=P)
                ).then_inc(dx, 16)
                s.wait_ge(vd, 37)  # y_f ready
                s.dma_start(
                    y_t[:].rearrange("(p t) one -> p (t one)", p=P), y_f[:]
                ).then_inc(dy, 16)
                s.wait_ge(dy, 16)

            @block.scalar
            def _(s):
                s.dma_start(co_sb[:], co_t[:]).then_inc(dc, 16)

            @block.tensor
            def _(t):
                t.wait_ge(vd, 2)  # hi2_f ready (implies consts loaded)
                t.transpose(out=hiT_ps[:], in_=hi2_f[:], identity=ident).then_inc(
                    ps, 1
                )
                t.wait_ge(vd, 6)  # hiT_sb (= hiT + q*64) ready
                t.matmul(rep_ps[:], lhsT=etile, rhs=hiT_sb[:]).then_inc(ps, 1)

            @block.gpsimd
            def _(g):
                # warm-up: the first dma_gather on a fresh NEFF stalls ~10us
                # (custom-op ucode IRAM load on all 8 Q7 cores). num_idxs=0
                # (all-negative idx) hangs the DMA, so keep 16 real indices.
                g.memset(widx[:], 0).then_inc(gw, 1)
                g.wait_ge(gw, 1)
                g.dma_gather(
                    out_ap=wout[:].rearrange("p (j k) -> p j k", k=BLK),
                    in_ap=lut_t[0][:],
                    idxs_ap=widx[:],
                    num_idxs=16,
                    num_idxs_reg=16,
                    elem_size=BLK,
                ).then_inc(gw, 16)
                g.wait_ge(vd, 7)  # idxw ready
                # queue c -> Q7 core pair (2c, 2c+1): descriptor generation
                # runs on four disjoint core pairs; each chunk is split in
                # two so the first half's data lands (and DVE select starts)
                # while the second half still generates.
                # any instruction on Q7 pair 0 blocks the next dispatch until
                # pair 0 finishes, so queue 0's pieces go dispatch-last
                for c, h in DORDER:
                    g.dma_gather(
                        out_ap=blocks4[:, c, 2 * h : 2 * h + 2],
                        in_ap=lut_t[c][:],
                        idxs_ap=idxw[:, c * 32 + 16 * h : c * 32 + 16 * h + 16],
                        num_idxs=CHUNK // 2,
                        num_idxs_reg=CHUNK // 2,
                        elem_size=BLK,
                        queue_num=c,
                    ).then_inc(gsl[c], 16)

            @block.vector
            def _(v):
                # the DVE pipeline is not hazard-safe for back-to-back
                # dependent ops: chain every op through sem `vd`
                n = [0]

                def step(inst):
                    inst.then_inc(vd, 1)
                    n[0] += 1

                def w():
                    if n[0]:
                        v.wait_ge(vd, n[0])

                v.wait_ge(dx, 16)
                v.wait_ge(dc, 16)
                # hi = addr>>8 directly from the high bits of x
                step(v.tensor_tensor(out=ph3, in0=x3[:, :, 8:14], in1=wh3, op=Alu.mult))
                w()
                step(v.tensor_reduce(out=hi2_f[:], in_=ph3, axis=X, op=Alu.add))
                # select-path arithmetic fills the PE-transpose latency
                step(v.tensor_tensor(out=pk3, in0=x3[:, :, 2:8], in1=w173, op=Alu.mult))
                w()
                step(v.tensor_reduce(out=k16_f[:], in_=pk3, axis=X, op=Alu.add))
                step(v.tensor_scalar(
                    out=tmp8[:],
                    in0=x3[:, :, 0:1].rearrange("p t one -> p (t one)"),
                    scalar1=8.0, scalar2=None, op0=Alu.mult,
                ))
                v.wait_ge(ps, 1)
                step(v.tensor_tensor(
                    out=hiT_sb[:], in0=hiT_ps[:], in1=qw16, op=Alu.add
                ))
                v.wait_ge(ps, 2)
                step(v.tensor_copy(out=idxw[:], in_=rep_ps[:]))
                # u32 select state: k32 = (addr>>2)&63, iota 0..63,
                # shmt = 8*(addr&3) for the final byte shift
                w()
                step(v.tensor_copy(out=k32_u[:], in_=k16_f[:]))
                step(v.tensor_copy(out=iota32[:], in_=iota[:, 0 : BLK // 4]))
                step(v.scalar_tensor_tensor(
                    out=shmt[:],
                    in0=x3[:, :, 1:2].rearrange("p t one -> p (t one)"),
                    scalar=16.0, in1=tmp8[:], op0=Alu.mult, op1=Alu.add,
                ))
                # masks don't depend on the gathers
                w()
                for c in range(NCHUNK):
                    kb = (
                        k32_4[:, c]
                        .rearrange("p j -> p j ()")
                        .to_broadcast([P, NCHUNK, BLK // 4])
                    )
                    step(v.tensor_tensor(
                        out=mask4[:, c], in0=iota32_b, in1=kb, op=Alu.not_equal
                    ))
                for c in range(NCHUNK):
                    w()
                    step(v.tensor_scalar(
                        out=mask4[:, c], in0=mask4[:, c], scalar1=1,
                        scalar2=None, op0=Alu.subtract,
                    ))
                for c, h in DORDER:
                        v.wait_ge(gsl[c], 16 * (h + 1))
                        w()
                        step(v.tensor_tensor(
                            out=msel4[:, c, 2 * h : 2 * h + 2],
                            in0=mask4[:, c, 2 * h : 2 * h + 2],
                            in1=blocks_i32[:, c, 2 * h : 2 * h + 2],
                            op=Alu.bitwise_and,
                        ))
                        w()
                        # one -1 mask per row selects its u32; OR-reduce is
                        # bit-exact for any int8 LUT content
                        step(v.tensor_reduce(
                            out=y32u[:, c * NCHUNK + 2 * h : c * NCHUNK + 2 * h + 2],
                            in_=msel4[:, c, 2 * h : 2 * h + 2],
                            axis=X, op=Alu.bitwise_or,
                        ))
                # byte extract + sign-extend, exact bitwise ops
                w()
                step(v.tensor_tensor(
                    out=sh_i[:], in0=y32u[:], in1=shmt[:],
                    op=Alu.logical_shift_right,
                ))
                w()
                step(v.tensor_scalar(
                    out=u8_i[:], in0=sh_i[:], scalar1=255, scalar2=128,
                    op0=Alu.bitwise_and, op1=Alu.bitwise_xor,
                ))
                w()
                step(v.tensor_scalar(
                    out=y_f[:], in0=u8_i[:], scalar1=128, scalar2=None,
                    op0=Alu.subtract,
                ))  # vd -> 37: y_f ready

    nc.compile()
    return nc


def _get_nc():
    if "nc" not in _CACHE:
        _CACHE["nc"] = _build_nc()
    return _CACHE["nc"]


def _consts() -> np.ndarray:
    co = np.zeros((P, NCOL), dtype=np.float32)
    co[:, 0:7] = 2.0 ** np.arange(0, 7, dtype=np.float32)  # 2^(b-1), b=1..7
    co[:, 7:13] = 2.0 ** np.arange(0, 6, dtype=np.float32)  # 2^(b-8), b=8..13
    co[:, 13:141] = np.arange(P, dtype=np.float32)[None, :]
    co[:, 141:269] = np.eye(P, dtype=np.float32)
    # qw16[qh, pi] = (j*128 + u*16 + qh) * 64 with pi = c*32 + j*8 + u
    pi = np.arange(P)
    j, u = (pi % 32) // 8, pi % 8
    co[:, 269:397] = (
        (j * P + u * 16)[None, :] + (np.arange(P) % 16)[:, None]
    ).astype(np.float32) * 64.0
    co[:16, 397:525] = (
        (np.arange(P)[None, :] % 16) == np.arange(16)[:, None]
    ).astype(np.float32)
    return co


def _make_in_maps(x, luts_int):
    co = _consts()
    x = np.asarray(x, dtype=np.float32).reshape(NUM_OUT, NUM_BITS)
    luts_int = np.asarray(luts_int, dtype=np.int8)
    in_maps = []
    for core in range(CORES):
        base = core * NS
        xl = x[base : base + NS]
        # combined layout [p, tau, b]: bits 0..7 from select-slot layout
        # (row tau*128+p), bits 8..13 from transpose layout (row p*16+tau)
        xs = np.empty((P, T, NUM_BITS), dtype=np.float32)
        xs[:, :, 0:8] = xl.reshape(T, P, NUM_BITS).transpose(1, 0, 2)[:, :, 0:8]
        xs[:, :, 8:14] = xl.reshape(P, T, NUM_BITS)[:, :, 8:14]
        m = {"x_shard": xs.reshape(NS, NUM_BITS), "consts": co}
        for c in range(NCHUNK):
            m[f"lut{c}"] = luts_int[
                base + c * CHUNK : base + (c + 1) * CHUNK
            ].reshape(NBLK, BLK)
        in_maps.append(m)
    return in_maps


def kernel(x, luts_float, luts_int, _run_kwargs=None):
    from concourse.bass_utils import run_bass_kernel_spmd

    nc = _get_nc()
    in_maps = _make_in_maps(x, luts_int)
    res = run_bass_kernel_spmd(nc, in_maps, list(range(CORES)), **(_run_kwargs or {}))
    _CACHE["last_result"] = res
    out = np.empty((NUM_OUT, 1), dtype=np.float32)
    for core in range(CORES):
        ys = res.results[core]["y_shard"].reshape(P, T)  # [p, t]
        out[core * NS : (core + 1) * NS, 0] = ys.T.reshape(NS)
    return out



# revision 28
# speedup vs baseline: 1.2174x; 1.1991x over previous
"""Trainium2 Bass kernel for nn_BinaryLutLayer (embedding_lookup).

Per output row n (of 16384): addr = sum_b x[n,b] * 2^b (14 bits), then
y[n] = float32(luts_int[n, addr]).

Sharding: rows split across 8 cores (2048 rows each), no communication.

Per-core pipeline (raw Bass, hand-scheduled across 5 engines):
  DVE:    block index (addr>>8) and u32-slot index (addr>>2)&63 as
          bit-weighted reduces of x; -1/0 masks (not_equal minus 1);
          per-piece bitwise_and select + bitwise_or reduce of the
          gathered u32 (bit-exact for any int8 LUT); final byte via
          logical shift by 8*(addr&3), mask, sign-fix.
  PE:     one transpose + one tiled-identity matmul land the int16
          block indices in the wrapped layout the gather firmware
          expects (partition q%16, col q//16, replicated to all 8
          gpsimd cores).
  Pool:   8x dma_gather of 256-byte LUT blocks (2048 descriptors
          total instead of reading the 32 MB LUT shard), spread over
          all 4 SWDGE queues = 4 disjoint Q7 core pairs so descriptor
          generation runs 4-way parallel. Two waves of <=256 indices
          per queue keep each lane inside the descriptor-ring window.
          Q7 pair 0 is ~2.2x slower and any instruction on it blocks
          the next dispatch until it finishes, so queue 0's two pieces
          are dispatched last. A 16-index dummy gather issued before
          the indices exist absorbs the ~10us mlp-library IRAM load.
  SP/ACT: input/output DMAs on parallel HWDGE queues.

The host does layout-only work: one combined x tensor (bits 0..7 in
select-slot layout, bits 8..13 in transpose-friendly layout), LUT chunk
slicing, and un-permuting y. All device arithmetic is exact: fp32 adds
below 2^24 for the index math, pure bitwise ops for the select.
"""

import numpy as np

NUM_BITS = 14
NUM_OUT = 16384
LUT_SIZE = 2**NUM_BITS
CORES = 8
NS = NUM_OUT // CORES  # rows per core = 2048
P = 128  # SBUF partitions
T = NS // P  # row-slots per partition = 16
NCHUNK = 4
CHUNK = NS // NCHUNK  # rows per dma_gather = 512
BLK = 256  # gather element size (bytes)
NBLK = CHUNK * (LUT_SIZE // BLK)  # blocks per LUT chunk = 32768
NCOL = 525  # consts columns

_CACHE: dict = {}


def _build_nc():
    import concourse.bacc as bacc
    from concourse import bass, mybir

    f32, i32, i16, i8, u16, u32 = (
        mybir.dt.float32,
        mybir.dt.int32,
        mybir.dt.int16,
        mybir.dt.int8,
        mybir.dt.uint16,
        mybir.dt.uint32,
    )
    Alu = mybir.AluOpType
    X = mybir.AxisListType.X

    nc = bacc.Bacc(
        "TRN2",
        target_bir_lowering=False,
        debug=False,
        dynamic_dma_scratch_size=65536,
        num_swdge_queues=4,
    )

    x_t = nc.dram_tensor("x_shard", [NS, NUM_BITS], f32, kind="ExternalInput")
    lut_t = [
        nc.dram_tensor(f"lut{c}", [NBLK, BLK], i8, kind="ExternalInput")
        for c in range(NCHUNK)
    ]
    co_t = nc.dram_tensor("consts", [P, NCOL], f32, kind="ExternalInput")
    y_t = nc.dram_tensor("y_shard", [NS, 1], f32, kind="ExternalOutput")

    from contextlib import ExitStack

    with ExitStack() as ctx:
        dx = ctx.enter_context(nc.semaphore("dx"))
        dc = ctx.enter_context(nc.semaphore("dc"))
        vd = ctx.enter_context(nc.semaphore("vd"))
        ps = ctx.enter_context(nc.semaphore("psem"))
        gsl = [ctx.enter_context(nc.semaphore(f"gs{i}")) for i in range(NCHUNK)]
        gw = ctx.enter_context(nc.semaphore("gw"))
        dy = ctx.enter_context(nc.semaphore("dy"))
        widx = ctx.enter_context(nc.sbuf_tensor("widx", [P, 1], i16))
        wout = ctx.enter_context(nc.sbuf_tensor("wout", [P, BLK], i8))
        x_sb = ctx.enter_context(nc.sbuf_tensor("x_sb", [P, T * NUM_BITS], f32))
        co_sb = ctx.enter_context(nc.sbuf_tensor("co_sb", [P, NCOL], f32))
        prodh = ctx.enter_context(nc.sbuf_tensor("prodh", [P, T * 6], f32))
        prodk = ctx.enter_context(nc.sbuf_tensor("prodk", [P, T * 6], f32))
        hi2_f = ctx.enter_context(nc.sbuf_tensor("hi2_f", [P, T], f32))
        hiT_ps = ctx.enter_context(nc.psum_tensor("hiT_ps", [16, P], f32))
        hiT_sb = ctx.enter_context(nc.sbuf_tensor("hiT_sb", [16, P], f32))
        rep_ps = ctx.enter_context(nc.psum_tensor("rep_ps", [P, P], f32))
        idxw = ctx.enter_context(nc.sbuf_tensor("idxw", [P, P], i16))
        blocks = ctx.enter_context(nc.sbuf_tensor("blocks", [P, T * BLK], i8))
        k16_f = ctx.enter_context(nc.sbuf_tensor("k16_f", [P, T], f32))
        k32_u = ctx.enter_context(nc.sbuf_tensor("k32_u", [P, T], u32))
        iota32 = ctx.enter_context(nc.sbuf_tensor("iota32", [P, BLK // 4], u32))
        tmp8 = ctx.enter_context(nc.sbuf_tensor("tmp8", [P, T], i32))
        shmt = ctx.enter_context(nc.sbuf_tensor("shmt", [P, T], i32))
        mask = ctx.enter_context(nc.sbuf_tensor("mask", [P, T * (BLK // 4)], i32))
        msel = ctx.enter_context(nc.sbuf_tensor("msel", [P, T * (BLK // 4)], i32))
        y32u = ctx.enter_context(nc.sbuf_tensor("y32u", [P, T], i32))
        sh_i = ctx.enter_context(nc.sbuf_tensor("sh_i", [P, T], i32))
        u8_i = ctx.enter_context(nc.sbuf_tensor("u8_i", [P, T], i32))
        y_f = ctx.enter_context(nc.sbuf_tensor("y_f", [P, T], f32))
        w17 = co_sb[:, 0:7]  # 2^(b-1), b=1..7
        wh = co_sb[:, 7:13]  # 2^(b-8), b=8..13
        iota = co_sb[:, 13:141]  # value k, f32
        ident = co_sb[:, 141:269]
        qw16 = co_sb[0:16, 269:397]  # q*64 at its wrap position
        etile = co_sb[0:16, 397:525]  # E[k, m] = (m%16 == k)

        x3 = x_sb[:].rearrange("p (t b) -> p t b", b=NUM_BITS)
        ph3 = prodh[:].rearrange("p (t b) -> p t b", b=6)
        pk3 = prodk[:].rearrange("p (t b) -> p t b", b=6)
        wh3 = wh.rearrange("p b -> p () b").to_broadcast([P, T, 6])
        w173 = w17[:, 0:6].rearrange("p b -> p () b").to_broadcast([P, T, 6])
        blocks4 = blocks[:].rearrange("p (c j k) -> p c j k", c=NCHUNK, k=BLK)
        blocks_i32 = blocks[:].bitcast(i32).rearrange(
            "p (c j k) -> p c j k", c=NCHUNK, k=BLK // 4
        )
        mask4 = mask[:].rearrange("p (c j k) -> p c j k", c=NCHUNK, k=BLK // 4)
        msel4 = msel[:].rearrange("p (c j k) -> p c j k", c=NCHUNK, k=BLK // 4)
        iota32_b = iota32[:].rearrange("p k -> p () k").to_broadcast(
            [P, NCHUNK, BLK // 4]
        )
        k32_4 = k32_u[:].rearrange("p (c j) -> p c j", c=NCHUNK)
        # dispatch order: a gather whose Q7 core pair differs from the
        # previous gather's pair dispatches ~instantly, so queue 0 (the
        # warmup's pair) goes last and all four generations overlap.
        CORDER = [1, 2, 3, 0]
        # (chunk/queue, half) dispatch order; queue 0 last
        DORDER = [(1, 0), (2, 0), (3, 0), (1, 1), (2, 1), (3, 1), (0, 0), (0, 1)]

        with nc.Block(no_gpsimd_drain=False) as block:

            @block.sync
            def _(s):
                s.dma_start(
                    x_sb[:], x_t[:].rearrange("(p t) b -> p (t b)", p=P)
                ).then_inc(dx, 16)
                s.wait_ge(vd, 37)  # y_f ready
                s.dma_start(
                    y_t[:].rearrange("(p t) one -> p (t one)", p=P), y_f[:]
                ).then_inc(dy, 16)
                s.wait_ge(dy, 16)

            @block.scalar
            def _(s):
                s.dma_start(co_sb[:], co_t[:]).then_inc(dc, 16)

            @block.tensor
            def _(t):
                t.wait_ge(vd, 2)  # hi2_f ready (implies consts loaded)
                t.transpose(out=hiT_ps[:], in_=hi2_f[:], identity=ident).then_inc(
                    ps, 1
                )
                t.wait_ge(vd, 6)  # hiT_sb (= hiT + q*64) ready
                t.matmul(rep_ps[:], lhsT=etile, rhs=hiT_sb[:]).then_inc(ps, 1)

            @block.gpsimd
            def _(g):
                # warm-up: the first dma_gather on a fresh NEFF stalls ~10us
                # (custom-op ucode IRAM load on all 8 Q7 cores). num_idxs=0
                # (all-negative idx) hangs the DMA, so keep 16 real indices.
                g.memset(widx[:], 0).then_inc(gw, 1)
                g.wait_ge(gw, 1)
                g.dma_gather(
                    out_ap=wout[:].rearrange("p (j k) -> p j k", k=BLK),
                    in_ap=lut_t[0][:],
                    idxs_ap=widx[:],
                    num_idxs=16,
                    num_idxs_reg=16,
                    elem_size=BLK,
                ).then_inc(gw, 16)
                g.wait_ge(vd, 7)  # idxw ready
                # queue c -> Q7 core pair (2c, 2c+1): descriptor generation
                # runs on four disjoint core pairs; each chunk is split in
                # two so the first half's data lands (and DVE select starts)
                # while the second half still generates.
                # any instruction on Q7 pair 0 blocks the next dispatch until
                # pair 0 finishes, so queue 0's pieces go dispatch-last
                for c, h in DORDER:
                    g.dma_gather(
                        out_ap=blocks4[:, c, 2 * h : 2 * h + 2],
                        in_ap=lut_t[c][:],
                        idxs_ap=idxw[:, c * 32 + 16 * h : c * 32 + 16 * h + 16],
                        num_idxs=CHUNK // 2,
                        num_idxs_reg=CHUNK // 2,
                        elem_size=BLK,
                        queue_num=c,
                    ).then_inc(gsl[c], 16)

            @block.vector
            def _(v):
                # the DVE pipeline is not hazard-safe for back-to-back
                # dependent ops: chain every op through sem `vd`
                n = [0]

                def step(inst):
                    inst.then_inc(vd, 1)
                    n[0] += 1

                def w():
                    if n[0]:
                        v.wait_ge(vd, n[0])

                v.wait_ge(dx, 16)
                v.wait_ge(dc, 16)
                # hi = addr>>8 directly from the high bits of x
                step(v.tensor_tensor(out=ph3, in0=x3[:, :, 8:14], in1=wh3, op=Alu.mult))
                w()
                step(v.tensor_reduce(out=hi2_f[:], in_=ph3, axis=X, op=Alu.add))
                # select-path arithmetic fills the PE-transpose latency
                step(v.tensor_tensor(out=pk3, in0=x3[:, :, 2:8], in1=w173, op=Alu.mult))
                w()
                step(v.tensor_reduce(out=k16_f[:], in_=pk3, axis=X, op=Alu.add))
                step(v.tensor_scalar(
                    out=tmp8[:],
                    in0=x3[:, :, 0:1].rearrange("p t one -> p (t one)"),
                    scalar1=8.0, scalar2=None, op0=Alu.mult,
                ))
                v.wait_ge(ps, 1)
                step(v.tensor_tensor(
                    out=hiT_sb[:], in0=hiT_ps[:], in1=qw16, op=Alu.add
                ))
                v.wait_ge(ps, 2)
                step(v.tensor_copy(out=idxw[:], in_=rep_ps[:]))
                # u32 select state: k32 = (addr>>2)&63, iota 0..63,
                # shmt = 8*(addr&3) for the final byte shift
                w()
                step(v.tensor_copy(out=k32_u[:], in_=k16_f[:]))
                step(v.tensor_copy(out=iota32[:], in_=iota[:, 0 : BLK // 4]))
                step(v.scalar_tensor_tensor(
                    out=shmt[:],
                    in0=x3[:, :, 1:2].rearrange("p t one -> p (t one)"),
                    scalar=16.0, in1=tmp8[:], op0=Alu.mult, op1=Alu.add,
                ))
                # masks don't depend on the gathers
                w()
                for c in range(NCHUNK):
                    kb = (
                        k32_4[:, c]
                        .rearrange("p j -> p j ()")
                        .to_broadcast([P, NCHUNK, BLK // 4])
                    )
                    step(v.tensor_tensor(
                        out=mask4[:, c], in0=iota32_b, in1=kb, op=Alu.not_equal
                    ))
                for c in range(NCHUNK):
                    w()
                    step(v.tensor_scalar(
                        out=mask4[:, c], in0=mask4[:, c], scalar1=1,
                        scalar2=None, op0=Alu.subtract,
                    ))
                for c, h in DORDER:
                        v.wait_ge(gsl[c], 16 * (h + 1))
                        w()
                        step(v.tensor_tensor(
                            out=msel4[:, c, 2 * h : 2 * h + 2],
                            in0=mask4[:, c, 2 * h : 2 * h + 2],
                            in1=blocks_i32[:, c, 2 * h : 2 * h + 2],
                            op=Alu.bitwise_and,
                        ))
                        w()
                        # one -1 mask per row selects its u32; OR-reduce is
                        # bit-exact for any int8 LUT content
                        step(v.tensor_reduce(
                            out=y32u[:, c * NCHUNK + 2 * h : c * NCHUNK + 2 * h + 2],
                            in_=msel4[:, c, 2 * h : 2 * h + 2],
                            axis=X, op=Alu.bitwise_or,
                        ))
                # byte extract + sign-extend, exact bitwise ops
                w()
                step(v.tensor_tensor(
                    out=sh_i[:], in0=y32u[:], in1=shmt[:],
                    op=Alu.logical_shift_right,
                ))
                w()
                step(v.tensor_scalar(
                    out=u8_i[:], in0=sh_i[:], scalar1=255, scalar2=128,
                    op0=Alu.bitwise_and, op1=Alu.bitwise_xor,
                ))
                w()
                step(v.tensor_scalar(
                    out=y_f[:], in0=u8_i[:], scalar1=128, scalar2=None,
                    op0=Alu.subtract,
                ))  # vd -> 37: y_f ready

    nc.compile()
    return nc


def _get_nc():
    if "nc" not in _CACHE:
        _CACHE["nc"] = _build_nc()
    return _CACHE["nc"]


def _consts() -> np.ndarray:
    co = np.zeros((P, NCOL), dtype=np.float32)
    co[:, 0:7] = 2.0 ** np.arange(0, 7, dtype=np.float32)  # 2^(b-1), b=1..7
    co[:, 7:13] = 2.0 ** np.arange(0, 6, dtype=np.float32)  # 2^(b-8), b=8..13
    co[:, 13:141] = np.arange(P, dtype=np.float32)[None, :]
    co[:, 141:269] = np.eye(P, dtype=np.float32)
    # qw16[qh, pi] = (j*128 + u*16 + qh) * 64 with pi = c*32 + j*8 + u
    pi = np.arange(P)
    j, u = (pi % 32) // 8, pi % 8
    co[:, 269:397] = (
        (j * P + u * 16)[None, :] + (np.arange(P) % 16)[:, None]
    ).astype(np.float32) * 64.0
    co[:16, 397:525] = (
        (np.arange(P)[None, :] % 16) == np.arange(16)[:, None]
    ).astype(np.float32)
    return co


def _make_in_maps(x, luts_int):
    co = _consts()
    x = np.asarray(x, dtype=np.float32).reshape(NUM_OUT, NUM_BITS)
    luts_int = np.asarray(luts_int, dtype=np.int8)
    in_maps = []
    for core in range(CORES):
        base = core * NS
        xl = x[base : base + NS]
        # combined layout [p, tau, b]: bits 0..7 from select-slot layout
        # (row tau*128+p), bits 8..13 from transpose layout (row p*16+tau)
        xs = np.empty((P, T, NUM_BITS), dtype=np.float32)
        xs[:, :, 0:8] = xl.reshape(T, P, NUM_BITS).transpose(1, 0, 2)[:, :, 0:8]
        xs[:, :, 8:14] = xl.reshape(P, T, NUM_BITS)[:, :, 8:14]
        m = {"x_shard": xs.reshape(NS, NUM_BITS), "consts": co}
        for c in range(NCHUNK):
            m[f"lut{c}"] = luts_int[
                base + c * CHUNK : base + (c + 1) * CHUNK
            ].reshape(NBLK, BLK)
        in_maps.append(m)
    return in_maps


def kernel(x, luts_float, luts_int, _run_kwargs=None):
    from concourse.bass_utils import run_bass_kernel_spmd

    nc = _get_nc()
    in_maps = _make_in_maps(x, luts_int)
    res = run_bass_kernel_spmd(nc, in_maps, list(range(CORES)), **(_run_kwargs or {}))
    _CACHE["last_result"] = res
    out = np.empty((NUM_OUT, 1), dtype=np.float32)
    for core in range(CORES):
        ys = res.results[core]["y_shard"].reshape(P, T)  # [p, t]
        out[core * NS : (core + 1) * NS, 0] = ys.T.reshape(NS)
    return out

